# revision 2
# baseline (speedup 1.0000x reference)
"""TopK sparse autoencoder forward pass on 8 TRN2 NeuronCores.

Strategy: data-parallel over the token batch (8192 rows -> 1024 rows/core,
zero collectives). Per core:
  1. encode: pre = (x - b_dec) @ W_enc.T + b_enc, computed as fp32r (FP22)
     matmuls at full PE rate; batch rows on partitions, features on the
     free dim; acts = relu(pre) streamed to an HBM scratch buffer.
  2. top-64 threshold per row: top-8 of each 256-wide feature group
     (DVE Max8) -> 1152 candidates/row; 8 rounds of max8 + match_replace
     extract the exact 64th-largest value t_b.
  3. decode: masked = acts * (acts >= t_b); x_hat^T = W_dec^T.T-contracted
     over features via PE-transposed masked tiles, + b_dec.
"""

import os
import numpy as np

from concourse import bass, mybir
from concourse import tile
from concourse.bass_utils import run_bass_kernel_spmd

F32 = mybir.dt.float32
F32R = mybir.dt.float32r

N_CORES = 8
B, D, F, K = 8192, 2304, 36864, 64

# tiling
PT = 128           # partition tile
FT = 512           # encode feature tile (matmul moving dim)
GRP = 256          # max8 candidate group width
FC = 128           # decode feature chunk (transpose tile)
SUP = 16           # decode feature chunks per super-chunk


def split_waits(nc, maxw=1):
    """Walrus in this container accepts few sync-waits per instruction; Tile
    emits many. Move excess waits onto standalone same-engine no-ops."""
    for fn in nc.m.functions:
        for blk in fn.blocks:
            newinsts = []
            for inst in blk.instructions:
                si = inst.sync_info
                if si is not None and len(si.on_wait) > maxw:
                    extra = si.on_wait[:-maxw]
                    keep = si.on_wait[-maxw:]
                    for j, w in enumerate(extra):
                        nop = mybir.InstNoOp(name=f"{inst.name}-wsplit{j}", ins=[], outs=[])
                        nop.engine = inst.engine
                        nop.sync_info = mybir.SyncInfo(on_wait=[w], on_update=[])
                        newinsts.append(nop)
                    si.on_wait = keep
                newinsts.append(inst)
            blk.instructions = newinsts


def build_nc(b_loc, d, f, mmdt=F32R):
    nbt = b_loc // PT          # batch tiles
    nd = d // PT               # contraction chunks (encode) / d tiles (decode)
    nft = f // FT              # encode feature tiles
    ngrp = f // GRP            # candidate groups
    ncand = ngrp * 8           # candidates per row
    nfc = f // FC              # decode feature chunks
    nsup = nfc // SUP          # decode super chunks
    assert ncand >= K
    n_half = max(1, b_loc // 512)   # decode moving-dim halves
    hw = min(512, b_loc)

    nc = bass.Bass()
    xT = nc.declare_dram_parameter("xT", [d, b_loc], F32, isOutput=False)
    wencT = nc.declare_dram_parameter("W_encT", [d, f], F32, isOutput=False)
    wdecT = nc.declare_dram_parameter("W_decT", [f, d], F32, isOutput=False)
    b_enc = nc.declare_dram_parameter("b_enc", [f], F32, isOutput=False)
    b_dec = nc.declare_dram_parameter("b_dec", [d], F32, isOutput=False)
    ident_in = nc.declare_dram_parameter("ident", [PT, PT], F32, isOutput=False)
    out = nc.declare_dram_parameter("out", [d, b_loc], F32, isOutput=True)

    wencT_r = wencT.rearrange("(a p) f -> p a f", p=PT)   # [128, nd, f]
    wdecT_r = wdecT.rearrange("(g p) e -> p g e", p=PT)   # [128, nfc, d]
    xT_r = xT.rearrange("(a p) b -> p a b", p=PT)         # [128, nd, b_loc]
    out_r = out.rearrange("(a p) b -> p a b", p=PT)
    bdec_r = b_dec.rearrange("(a p) -> p a", p=PT)        # [128, nd]

    with tile.TileContext(nc) as tc:
        with tc.tile_pool(name="persist", bufs=1) as pp, \
             tc.tile_pool(name="dram", bufs=1, space="DRAM") as dp:
            acts_dram = dp.tile([nbt, PT, f], F32, name="acts_dram")
            ident = pp.tile([PT, PT], F32)
            nc.sync.dma_start(out=ident[:, :], in_=ident_in[:, :])
            ones_st = pp.tile([1, PT], F32)
            nc.vector.memset(ones_st[:, :], 1.0)
            ones = pp.tile([1, PT], mmdt)
            nc.vector.tensor_copy(ones[:, :], ones_st[:, :])
            bdec_sb = pp.tile([PT, nd], F32)
            nc.sync.dma_start(out=bdec_sb[:, :], in_=bdec_r[:, :])
            bdec_neg = pp.tile([PT, nd], F32)
            nc.vector.tensor_scalar_mul(bdec_neg[:, :], bdec_sb[:, :], -1.0)
            t_sb = pp.tile([PT, nbt], F32)

            # ---------------- encode + candidate collection ----------------
            with tc.tile_pool(name="enc_x", bufs=nd) as xp, \
                 tc.tile_pool(name="enc_w", bufs=max(nd + 8, int(1.6 * nd))) as wp, \
                 tc.tile_pool(name="enc_cand", bufs=nbt) as cp, \
                 tc.tile_pool(name="enc_st", bufs=4) as sp, \
                 tc.tile_pool(name="enc_misc", bufs=2) as mp, \
                 tc.tile_pool(name="psum_e", bufs=2, space="PSUM") as pse:

                xs = []
                for a in range(nd):
                    xst = sp.tile([PT, b_loc], F32, tag="xst", name=f"xst{a}", bufs=2)
                    nc.sync.dma_start(out=xst[:, :], in_=xT_r[:, a, :])
                    # x - b_dec (per-partition scalar), rounded to fp32r
                    xt = xp.tile([PT, b_loc], mmdt, tag="xs", name=f"xs{a}")
                    nc.scalar.activation(
                        xt[:, :], xst[:, :], mybir.ActivationFunctionType.Identity,
                        bias=bdec_neg[:, a : a + 1],
                    )
                    xs.append(xt)

                cands = []
                for bt in range(nbt):
                    cands.append(cp.tile([PT, ncand], F32, tag="cand", name=f"cand{bt}"))

                for ft in range(nft):
                    f0 = ft * FT
                    ws = []
                    for a in range(nd):
                        wst = sp.tile([PT, FT], F32, tag="wst", name=f"wst{ft}_{a}", bufs=3)
                        nc.sync.dma_start(out=wst[:, :], in_=wencT_r[:, a, f0 : f0 + FT])
                        wt = wp.tile([PT, FT], mmdt, tag="wenc", name=f"wenc{ft}_{a}")
                        nc.vector.tensor_copy(wt[:, :], wst[:, :])
                        ws.append(wt)
                    bes = mp.tile([1, FT], F32, tag="bencs", name=f"bencs{ft}")
                    nc.sync.dma_start(out=bes[:, :], in_=b_enc.rearrange("(o x) -> o x", o=1)[:, f0 : f0 + FT])
                    be = mp.tile([1, FT], mmdt, tag="benc", name=f"benc{ft}")
                    nc.vector.tensor_copy(be[:, :], bes[:, :])

                    for bt in range(nbt):
                        ps = pse.tile([PT, FT], F32, tag="pse", name=f"pse{ft}_{bt}")
                        for a in range(nd):
                            nc.tensor.matmul(
                                ps[:, :],
                                lhsT=xs[a][:, bt * PT : (bt + 1) * PT],
                                rhs=ws[a][:, :],
                                start=(a == 0),
                                stop=False,
                            )
                        nc.tensor.matmul(
                            ps[:, :], lhsT=ones[:, :], rhs=be[:, :],
                            start=False, stop=True,
                        )
                        ast = sp.tile([PT, FT], F32, tag="ast", name=f"ast{ft}_{bt}")
                        nc.vector.tensor_scalar_max(ast[:, :], ps[:, :], 0.0)
                        for g in range(FT // GRP):
                            c0 = (ft * (FT // GRP) + g) * 8
                            nc.vector.max(
                                cands[bt][:, c0 : c0 + 8],
                                ast[:, g * GRP : (g + 1) * GRP],
                            )
                        nc.sync.dma_start(
                            out=acts_dram[bt, :, f0 : f0 + FT], in_=ast[:, :]
                        )

                # ---------------- exact top-64 threshold extraction ----------------
                for bt in range(nbt):
                    t64 = sp.tile([PT, 64], F32, tag="t64", name=f"t64_{bt}", bufs=2)
                    for r in range(8):
                        nc.vector.max(t64[:, r * 8 : r * 8 + 8], cands[bt][:, :])
                        if r < 7:
                            nc.vector.match_replace(
                                cands[bt][:, :],
                                t64[:, r * 8 : r * 8 + 8],
                                cands[bt][:, :],
                                -1e30,
                            )
                    nc.vector.tensor_copy(t_sb[:, bt : bt + 1], t64[:, 63:64])

            # ---------------- decode ----------------
            with tc.tile_pool(name="dec_acc", bufs=nd) as accp, \
                 tc.tile_pool(name="dec_mt", bufs=SUP) as mtp, \
                 tc.tile_pool(name="dec_a", bufs=2) as dap, \
                 tc.tile_pool(name="dec_g", bufs=2) as dgp, \
                 tc.tile_pool(name="dec_w", bufs=2) as dwp, \
                 tc.tile_pool(name="psum_d", bufs=2, space="PSUM") as psd, \
                 tc.tile_pool(name="psum_t", bufs=2, space="PSUM") as pst:

                accs = [accp.tile([PT, b_loc], F32, tag="acc", name=f"acc{i}") for i in range(nd)]

                for sup in range(nsup):
                    fs0 = sup * SUP * FC
                    mts = []
                    for fc in range(SUP):
                        mts.append(mtp.tile([PT, b_loc], mmdt, tag="mt", name=f"mt{sup}_{fc}"))
                    for bt in range(nbt):
                        araw = dap.tile([PT, SUP * FC], F32, tag="araw", name=f"araw{sup}_{bt}")
                        nc.sync.dma_start(
                            out=araw[:, :],
                            in_=acts_dram[bt, :, fs0 : fs0 + SUP * FC],
                        )
                        # masked = (acts >= t) * acts in one DVE op
                        nc.vector.scalar_tensor_tensor(
                            araw[:, :], araw[:, :], t_sb[:, bt : bt + 1], araw[:, :],
                            mybir.AluOpType.is_ge, mybir.AluOpType.mult,
                        )
                        for fc in range(SUP):
                            pt_ = pst.tile([PT, PT], F32, tag="ptr", name=f"ptr{sup}_{bt}_{fc}")
                            nc.tensor.transpose(
                                pt_[:, :], araw[:, fc * FC : (fc + 1) * FC], ident[:, :]
                            )
                            nc.vector.tensor_copy(
                                mts[fc][:, bt * PT : (bt + 1) * PT], pt_[:, :]
                            )

                    for dt in range(nd):
                        wdst = dgp.tile([PT, SUP * PT], F32, tag="wdst", name=f"wdst{sup}_{dt}")
                        nc.sync.dma_start(
                            out=wdst.rearrange("p (c e) -> p c e", c=SUP)[:, :, :],
                            in_=wdecT_r[:, sup * SUP : (sup + 1) * SUP, dt * PT : (dt + 1) * PT],
                        )
                        wdr = dwp.tile([PT, SUP * PT], mmdt, tag="wdec", name=f"wdec{sup}_{dt}")
                        nc.vector.tensor_copy(wdr[:, :], wdst[:, :])
                        wds = [wdr[:, fc * PT : (fc + 1) * PT] for fc in range(SUP)]
                        ps2 = psd.tile([PT, b_loc], F32, tag="psd", name=f"psd{sup}_{dt}")
                        for h in range(n_half):
                            for fc in range(SUP):
                                nc.tensor.matmul(
                                    ps2[:, h * hw : (h + 1) * hw],
                                    lhsT=wds[fc],
                                    rhs=mts[fc][:, h * hw : (h + 1) * hw],
                                    start=(fc == 0),
                                    stop=(fc == SUP - 1),
                                )
                        if sup == 0:
                            nc.vector.tensor_copy(accs[dt][:, :], ps2[:, :])
                        else:
                            nc.vector.tensor_add(accs[dt][:, :], accs[dt][:, :], ps2[:, :])

                for dt in range(nd):
                    nc.scalar.activation(
                        accs[dt][:, :], accs[dt][:, :],
                        mybir.ActivationFunctionType.Identity,
                        bias=bdec_sb[:, dt : dt + 1],
                    )
                    nc.sync.dma_start(out=out_r[:, dt, :], in_=accs[dt][:, :])

    split_waits(nc)
    return nc


def kernel(x, W_enc, b_enc, W_dec, b_dec, mmdt=F32R):
    b, d = x.shape
    f = W_enc.shape[0]
    b_loc = b // N_CORES

    nc = build_nc(b_loc, d, f, mmdt)

    xT = np.ascontiguousarray(x.T.astype(np.float32))            # [d, b]
    wencT = np.ascontiguousarray(W_enc.T.astype(np.float32))     # [d, f]
    wdecT = np.ascontiguousarray(W_dec.T.astype(np.float32))     # [f, d]
    ident = np.eye(128, dtype=np.float32)
    in_maps = []
    for i in range(N_CORES):
        in_maps.append({
            "xT": np.ascontiguousarray(xT[:, i * b_loc : (i + 1) * b_loc]),
            "W_encT": wencT,
            "W_decT": wdecT,
            "b_enc": np.asarray(b_enc, dtype=np.float32),
            "b_dec": np.asarray(b_dec, dtype=np.float32),
            "ident": ident,
        })

    trace = bool(os.environ.get("BASS_TOPK_TRACE"))
    res = run_bass_kernel_spmd(nc, in_maps, list(range(N_CORES)), trace=trace)
    if trace and res.exec_time_ns is not None:
        print(f"HW exec time: {res.exec_time_ns} ns")
        if res.instructions_and_trace is not None:
            print(f"trace path: {res.instructions_and_trace[1]}")
        if res.profile_json is not None:
            print(f"profile json: {res.profile_json}")
    shards = [res.results[i]["out"] for i in range(N_CORES)]     # [d, b_loc] each
    xhatT = np.concatenate(shards, axis=1)                        # [d, b]
    return np.ascontiguousarray(xhatT.T)


if __name__ == "__main__":
    # small smoke config vs numpy simulation of the same math
    b_loc, d, f = 256, 256, 2048
    rng = np.random.default_rng(0)
    x = rng.standard_normal((N_CORES * b_loc, d), dtype=np.float32)
    W_enc = (rng.standard_normal((f, d), dtype=np.float32) / np.sqrt(d)).astype(np.float32)
    b_enc_ = rng.standard_normal(f, dtype=np.float32) * 0.01
    W_dec = rng.standard_normal((d, f), dtype=np.float32).astype(np.float32)
    b_dec_ = rng.standard_normal(d, dtype=np.float32) * 0.01

    import sys
    mmdt = F32 if "f32" in sys.argv[1:] else F32R
    got = kernel(x, W_enc, b_enc_, W_dec, b_dec_, mmdt)

    pre = (x - b_dec_) @ W_enc.T + b_enc_
    acts = np.maximum(pre, 0)
    # simulate the kernel's group-candidate threshold algorithm
    g = acts.reshape(acts.shape[0], -1, 256)
    cand = -np.sort(-g, axis=2)[:, :, :8].reshape(acts.shape[0], -1)
    kth = -np.sort(-cand, axis=1)[:, K - 1]
    masked = acts * (acts >= kth[:, None])
    want = masked @ W_dec.T + b_dec_
    err = np.linalg.norm(got - want) / np.linalg.norm(want)
    print("smoke rel err:", err)



# revision 14
# speedup vs baseline: 1.4213x; 1.4213x over previous
"""TopK sparse autoencoder forward pass on 8 TRN2 NeuronCores.

Strategy: data-parallel over the token batch (8192 rows -> 1024 rows/core,
zero collectives). Per core:
  1. encode: pre = (x - b_dec) @ W_enc.T + b_enc as fp32r matmuls at full PE
     rate (batch rows on partitions, features on the free dim), ReLU fused
     into the PSUM->SBUF copy on the Activation engine.
  2. top-64 with indices: per 256-wide feature group, DVE Max8 + MaxIndex
     give the top-8 values and their within-group positions; the position is
     packed into the low 8 mantissa bits of the value (2^-15 relative
     perturbation, far below fp32r noise). 8 rounds of Max8 + MaxIndex +
     MatchReplace over the 1152 packed candidates yield the top-64 packed
     values and their candidate positions; the feature index is recovered
     arithmetically: feat = 256*(pos>>3) | (bits(val) & 0xFF).
  3. sparse decode: for each of the 64 slots, an indirect DMA gathers
     W_dec^T rows (bf16) by feature index, one row per token partition;
     DVE scalar_tensor_tensor accumulates val_k * W_row into the output.
     No dense decode GEMM, no activation round-trip through HBM.
"""

import os
import numpy as np
import ml_dtypes

from concourse import bass, mybir
from concourse import tile
from concourse.bass_utils import run_bass_kernel_spmd

F32 = mybir.dt.float32
F32R = mybir.dt.float32r
U32 = mybir.dt.uint32
U16 = mybir.dt.uint16
BF16 = mybir.dt.bfloat16

N_CORES = 8
B, D, F, K = 8192, 2304, 36864, 64

PT = 128           # partition tile
FT = 512           # encode feature tile (matmul moving dim)
GRP = 256          # max8 candidate group width


def split_waits(nc, maxw=1):
    """Walrus in this container accepts few sync-waits per instruction; Tile
    emits many. Move excess waits onto standalone same-engine no-ops."""
    for fn in nc.m.functions:
        for blk in fn.blocks:
            newinsts = []
            for inst in blk.instructions:
                si = inst.sync_info
                if si is not None and len(si.on_wait) > maxw:
                    extra = si.on_wait[:-maxw]
                    keep = si.on_wait[-maxw:]
                    for j, w in enumerate(extra):
                        nop = mybir.InstNoOp(name=f"{inst.name}-wsplit{j}", ins=[], outs=[])
                        nop.engine = inst.engine
                        nop.sync_info = mybir.SyncInfo(on_wait=[w], on_update=[])
                        newinsts.append(nop)
                    si.on_wait = keep
                newinsts.append(inst)
            blk.instructions = newinsts


def build_nc(b_loc, d, f):
    nbt = b_loc // PT          # batch tiles
    nd = d // PT               # contraction chunks
    nft = f // FT              # encode feature tiles
    ngrp = f // GRP            # candidate groups per row
    ncand = ngrp * 8           # candidates per row
    assert ncand >= K and K % 8 == 0
    nrounds = K // 8

    nc = bass.Bass()
    # x and weights declared float32r: same 4-byte container as f32, PE
    # rounds internally — lets plain DMAs satisfy the BIR fp32r-producer
    # rule with zero cast instructions. The x - b_dec subtraction is folded
    # into b_enc host-side (b_enc - W_enc @ b_dec).
    xT = nc.declare_dram_parameter("xT", [d, b_loc], F32R, isOutput=False)
    wencT = nc.declare_dram_parameter("W_encT", [d, f], F32R, isOutput=False)
    wdec16 = nc.declare_dram_parameter("Wdec16", [f, d], BF16, isOutput=False)
    b_enc = nc.declare_dram_parameter("b_enc", [f], F32R, isOutput=False)
    bdecb = nc.declare_dram_parameter("bdecb", [PT, d], F32, isOutput=False)
    out = nc.declare_dram_parameter("out", [b_loc, d], F32, isOutput=True)

    wencT_r = wencT.rearrange("(a p) f -> p a f", p=PT)   # [128, nd, f]
    xT_r = xT.rearrange("(a p) b -> p a b", p=PT)         # [128, nd, b_loc]
    out_r = out.rearrange("(t p) e -> t p e", p=PT)       # [nbt, 128, d]

    with tile.TileContext(nc) as tc:
        with tc.tile_pool(name="persist", bufs=1) as pp:
            ones_st = pp.tile([1, PT], F32)
            nc.vector.memset(ones_st[:, :], 1.0)
            ones = pp.tile([1, PT], F32R)
            nc.vector.tensor_copy(ones[:, :], ones_st[:, :])
            bdecb_sb = pp.tile([PT, d], F32)
            nc.sync.dma_start(out=bdecb_sb[:, :], in_=bdecb[:, :])

            # x tiles, resident for the whole encode
            xs = []
            for a in range(nd):
                xt = pp.tile([PT, b_loc], F32R, name=f"xs{a}")
                nc.sync.dma_start(out=xt[:, :], in_=xT_r[:, a, :])
                xs.append(xt)

            candV = [pp.tile([PT, ncand], F32, name=f"candV{bt}") for bt in range(nbt)]
            candR = [pp.tile([PT, ncand], U16, name=f"candR{bt}") for bt in range(nbt)]
            t64 = [pp.tile([PT, K], F32, name=f"t64_{bt}") for bt in range(nbt)]
            pos = [pp.tile([PT, K], U32, name=f"pos{bt}") for bt in range(nbt)]
            feat = [pp.tile([PT, K], U32, name=f"feat{bt}") for bt in range(nbt)]

            # ---------------- encode + candidate collection ----------------
            with tc.tile_pool(name="enc_w", bufs=nd + 8) as wp, \
                 tc.tile_pool(name="enc_b", bufs=2) as bp, \
                 tc.tile_pool(name="enc_ast", bufs=4) as ap_, \
                 tc.tile_pool(name="psum_e", bufs=4, space="PSUM") as pse:

                for ft in range(nft):
                    f0 = ft * FT
                    ws = []
                    for a in range(nd):
                        wst = wp.tile([PT, FT], F32R, tag="wst", name=f"wst{ft}_{a}")
                        nc.sync.dma_start(out=wst[:, :], in_=wencT_r[:, a, f0 : f0 + FT])
                        ws.append(wst)
                    bes = bp.tile([1, FT], F32R, tag="bes", name=f"bes{ft}")
                    nc.sync.dma_start(
                        out=bes[:, :],
                        in_=b_enc.rearrange("(o x) -> o x", o=1)[:, f0 : f0 + FT],
                    )

                    for bt in range(nbt):
                        ps = pse.tile([PT, FT], F32, tag="pse", name=f"pse{ft}_{bt}")
                        for a in range(nd):
                            nc.tensor.matmul(
                                ps[:, :],
                                lhsT=xs[a][:, bt * PT : (bt + 1) * PT],
                                rhs=ws[a][:, :],
                                start=(a == 0),
                                stop=False,
                            )
                        nc.tensor.matmul(
                            ps[:, :], lhsT=ones[:, :],
                            rhs=bes[:, :],
                            start=False, stop=True,
                        )
                        ast = ap_.tile([PT, FT], F32, tag="ast", name=f"ast{ft}_{bt}")
                        nc.scalar.activation(
                            ast[:, :], ps[:, :], mybir.ActivationFunctionType.Relu
                        )
                        for g in range(FT // GRP):
                            c0 = (ft * (FT // GRP) + g) * 8
                            nc.vector.max(
                                candV[bt][:, c0 : c0 + 8],
                                ast[:, g * GRP : (g + 1) * GRP],
                            )
                            nc.vector.max_index(
                                candR[bt][:, c0 : c0 + 8],
                                candV[bt][:, c0 : c0 + 8],
                                ast[:, g * GRP : (g + 1) * GRP],
                            )

            # ---------------- pack + top-64 extraction ----------------
            with tc.tile_pool(name="extr", bufs=4) as ep:
                for bt in range(nbt):
                    candPu = candV[bt][:, :].bitcast(U32)
                    nc.vector.tensor_scalar(
                        candPu, candPu, 0xFFFFFF00, None, mybir.AluOpType.bitwise_and
                    )
                    cr32 = ep.tile([PT, ncand], U32, tag="cr32", name=f"cr32_{bt}")
                    nc.vector.tensor_copy(cr32[:, :], candR[bt][:, :])
                    nc.vector.tensor_tensor(
                        candPu, candPu, cr32[:, :], mybir.AluOpType.bitwise_or
                    )
                    for r in range(nrounds):
                        t8 = t64[bt][:, r * 8 : (r + 1) * 8]
                        nc.vector.max(t8, candV[bt][:, :])
                        nc.vector.max_index(
                            pos[bt][:, r * 8 : (r + 1) * 8], t8, candV[bt][:, :]
                        )
                        if r < nrounds - 1:
                            nc.vector.match_replace(
                                candV[bt][:, :], t8, candV[bt][:, :], -1e30
                            )
                    # feat = ((pos & ~7) << 5) | (bits(t64) & 0xFF)
                    nc.vector.tensor_scalar(
                        feat[bt][:, :], pos[bt][:, :], 0xFFFFFFF8, 5,
                        mybir.AluOpType.bitwise_and,
                        mybir.AluOpType.logical_shift_left,
                    )
                    lowt = ep.tile([PT, K], U32, tag="lowt", name=f"lowt{bt}")
                    nc.vector.tensor_scalar(
                        lowt[:, :], t64[bt][:, :].bitcast(U32), 0xFF, None,
                        mybir.AluOpType.bitwise_and,
                    )
                    nc.vector.tensor_tensor(
                        feat[bt][:, :], feat[bt][:, :], lowt[:, :],
                        mybir.AluOpType.bitwise_or,
                    )

            # ---------------- sparse gather decode ----------------
            with tc.tile_pool(name="dec_g", bufs=8) as gp, \
                 tc.tile_pool(name="dec_acc", bufs=3) as accp:
                for bt in range(nbt):
                    acc = accp.tile([PT, d], F32, tag="acc", name=f"acc{bt}")
                    for k in range(K):
                        G = gp.tile([PT, d], BF16, tag="g", name=f"g{bt}_{k}")
                        nc.gpsimd.indirect_dma_start(
                            out=G[:, :],
                            out_offset=None,
                            in_=wdec16[:, :],
                            in_offset=bass.IndirectOffsetOnAxis(
                                ap=feat[bt][:, k : k + 1], axis=0
                            ),
                        )
                        nc.vector.scalar_tensor_tensor(
                            acc[:, :], G[:, :], t64[bt][:, k : k + 1],
                            bdecb_sb[:, :] if k == 0 else acc[:, :],
                            mybir.AluOpType.mult, mybir.AluOpType.add,
                        )
                    nc.sync.dma_start(out=out_r[bt, :, :], in_=acc[:, :])

    split_waits(nc)
    return nc


def kernel(x, W_enc, b_enc, W_dec, b_dec):
    b, d = x.shape
    f = W_enc.shape[0]
    b_loc = b // N_CORES

    nc = build_nc(b_loc, d, f)

    xT = np.ascontiguousarray(np.asarray(x, dtype=np.float32).T)       # [d, b]
    wenc = np.asarray(W_enc, dtype=np.float32)
    wencT = np.ascontiguousarray(wenc.T)                               # [d, f]
    wdec16 = np.ascontiguousarray(
        np.asarray(W_dec, dtype=np.float32).T.astype(ml_dtypes.bfloat16)
    )  # [f, d] bf16
    bdec = np.asarray(b_dec, dtype=np.float32)
    bdecb = np.ascontiguousarray(np.broadcast_to(bdec, (PT, d)))
    # fold the x - b_dec subtraction into the encoder bias
    benc_eff = np.asarray(b_enc, dtype=np.float32) - wenc @ bdec
    in_maps = []
    for i in range(N_CORES):
        in_maps.append({
            "xT": np.ascontiguousarray(xT[:, i * b_loc : (i + 1) * b_loc]),
            "W_encT": wencT,
            "Wdec16": wdec16,
            "b_enc": benc_eff,
            "bdecb": bdecb,
        })

    trace = bool(os.environ.get("BASS_TOPK_TRACE"))
    res = run_bass_kernel_spmd(nc, in_maps, list(range(N_CORES)), trace=trace)
    if trace and res.exec_time_ns is not None:
        print(f"HW exec time: {res.exec_time_ns} ns")
        if res.instructions_and_trace is not None:
            print(f"trace path: {res.instructions_and_trace[1]}")
        if res.profile_json is not None:
            print(f"profile json: {res.profile_json}")
    shards = [res.results[i]["out"] for i in range(N_CORES)]     # [b_loc, d] each
    return np.ascontiguousarray(np.concatenate(shards, axis=0))


if __name__ == "__main__":
    # small smoke config vs numpy simulation of the same math
    b_loc, d, f = 256, 256, 2048
    rng = np.random.default_rng(0)
    x = rng.standard_normal((N_CORES * b_loc, d), dtype=np.float32)
    W_enc = (rng.standard_normal((f, d), dtype=np.float32) / np.sqrt(d)).astype(np.float32)
    b_enc_ = rng.standard_normal(f, dtype=np.float32) * 0.01
    W_dec = rng.standard_normal((d, f), dtype=np.float32).astype(np.float32)
    b_dec_ = rng.standard_normal(d, dtype=np.float32) * 0.01

    got = kernel(x, W_enc, b_enc_, W_dec, b_dec_)

    pre = (x - b_dec_) @ W_enc.T + b_enc_
    acts = np.maximum(pre, 0)
    # simulate the kernel's group-candidate top-64 (with packed low bits)
    g = acts.reshape(acts.shape[0], -1, GRP)
    order = np.argsort(-g, axis=2, kind="stable")[:, :, :8]
    top8v = np.take_along_axis(g, order, axis=2)
    packed = ((top8v.view(np.uint32) & 0xFFFFFF00) | order.astype(np.uint32)).view(np.float32)
    flat = packed.reshape(acts.shape[0], -1)
    srt = np.argsort(-flat, axis=1, kind="stable")[:, :K]
    vals = np.take_along_axis(flat, srt, axis=1)
    feats = ((srt & ~7) << 5) | (vals.view(np.uint32) & 0xFF)
    wd16 = W_dec.T.astype(ml_dtypes.bfloat16).astype(np.float32)  # [f, d]
    want = np.einsum("bk,bkd->bd", vals, wd16[feats]) + b_dec_
    err = np.linalg.norm(got - want) / np.linalg.norm(want)
    print("smoke rel err:", err)


# revision 19
# speedup vs baseline: 1.5299x; 1.0764x over previous
"""TopK sparse autoencoder forward pass on 8 TRN2 NeuronCores.

Strategy: data-parallel over the token batch (8192 rows -> 1024 rows/core,
zero collectives). Per core, the batch is processed in two halves so the
second half's encode overlaps the first half's sparse decode:
  1. encode: pre = x @ W_enc.T + b_eff as fp32r matmuls at full PE rate
     (batch rows on partitions, features on the free dim); the x - b_dec
     subtraction is folded into b_eff host-side. ReLU fused into the
     PSUM->SBUF copy on the Activation engine.
  2. top-64 with indices: per 256-wide feature group, DVE Max8 + MaxIndex
     give the top-8 values and their within-group positions; the position is
     packed into the low 8 mantissa bits of the value (2^-15 relative
     perturbation, far below fp32r noise). 8 rounds of Max8 + MaxIndex +
     MatchReplace over the 1152 packed candidates yield the top-64 packed
     values and their candidate positions; the feature index is recovered
     arithmetically: feat = 256*(pos>>3) | (bits(val) & 0xFF).
  3. sparse decode: for each of the 64 slots, an indirect DMA gathers
     W_dec^T rows (bf16) by feature index, one row per token partition.
     Half A (overlapped with half B's encode): Act scales G*val -> fp16,
     Pool accumulates — DVE stays free for half B's candidate scan, and
     the ops are emission-interleaved into the encode loop to avoid
     head-of-line blocking on the in-order engine queues. The final half
     accumulates with DVE scalar_tensor_tensor (Pool is busy generating
     gather descriptors).
"""

import os
import numpy as np
import ml_dtypes

from concourse import bass, mybir
from concourse import tile
from concourse.bass_utils import run_bass_kernel_spmd

F32 = mybir.dt.float32
F32R = mybir.dt.float32r
F16 = mybir.dt.float16
U32 = mybir.dt.uint32
U16 = mybir.dt.uint16
BF16 = mybir.dt.bfloat16

N_CORES = 8
B, D, F, K = 8192, 2304, 36864, 64

PT = 128           # partition tile
FT = 512           # encode feature tile (matmul moving dim)
GRP = 256          # max8 candidate group width
N_HALF = 2         # batch halves for encode/decode overlap


def split_waits(nc, maxw=1):
    """Walrus in this container accepts few sync-waits per instruction; Tile
    emits many. Move excess waits onto standalone same-engine no-ops."""
    for fn in nc.m.functions:
        for blk in fn.blocks:
            newinsts = []
            for inst in blk.instructions:
                si = inst.sync_info
                if si is not None and len(si.on_wait) > maxw:
                    extra = si.on_wait[:-maxw]
                    keep = si.on_wait[-maxw:]
                    for j, w in enumerate(extra):
                        nop = mybir.InstNoOp(name=f"{inst.name}-wsplit{j}", ins=[], outs=[])
                        nop.engine = inst.engine
                        nop.sync_info = mybir.SyncInfo(on_wait=[w], on_update=[])
                        newinsts.append(nop)
                    si.on_wait = keep
                newinsts.append(inst)
            blk.instructions = newinsts


def build_nc(b_loc, d, f, with_bias, with_bdec):
    nbt = b_loc // PT          # batch tiles
    nd = d // PT               # contraction chunks
    nft = f // FT              # encode feature tiles
    ngrp = f // GRP            # candidate groups per row
    ncand = ngrp * 8           # candidates per row
    assert ncand >= K and K % 8 == 0
    nrounds = K // 8
    assert nbt % N_HALF == 0
    nbh = nbt // N_HALF        # batch tiles per half

    nc = bass.Bass()
    # x and weights declared float32r: same 4-byte container as f32, PE
    # rounds internally — lets plain DMAs satisfy the BIR fp32r-producer
    # rule with zero cast instructions.
    xT = nc.declare_dram_parameter("xT", [d, b_loc], F32R, isOutput=False)
    wencT = nc.declare_dram_parameter("W_encT", [d, f], F32R, isOutput=False)
    wdec16 = nc.declare_dram_parameter("Wdec16", [f, d], BF16, isOutput=False)
    if with_bias:
        b_enc = nc.declare_dram_parameter("b_enc", [f], F32R, isOutput=False)
    if with_bdec:
        bdecb = nc.declare_dram_parameter("bdecb", [PT, d], F32, isOutput=False)
    out = nc.declare_dram_parameter("out", [b_loc, d], F32, isOutput=True)

    wencT_r = wencT.rearrange("(a p) f -> p a f", p=PT)   # [128, nd, f]
    xT_r = xT.rearrange("(a p) b -> p a b", p=PT)         # [128, nd, b_loc]
    out_r = out.rearrange("(t p) e -> t p e", p=PT)       # [nbt, 128, d]

    with tile.TileContext(nc) as tc:
        with tc.tile_pool(name="persist", bufs=1) as pp:
            if with_bias:
                ones_st = pp.tile([1, PT], F32)
                nc.vector.memset(ones_st[:, :], 1.0)
                ones = pp.tile([1, PT], F32R)
                nc.vector.tensor_copy(ones[:, :], ones_st[:, :])
            if with_bdec:
                bdecb_sb = pp.tile([PT, d], F32)
                nc.sync.dma_start(out=bdecb_sb[:, :], in_=bdecb[:, :])

            # x tiles, resident for the whole encode
            xs = []
            for a in range(nd):
                xt = pp.tile([PT, b_loc], F32R, name=f"xs{a}")
                nc.sync.dma_start(out=xt[:, :], in_=xT_r[:, a, :])
                xs.append(xt)

            with tc.tile_pool(name="candV", bufs=nbh) as cvp, \
                 tc.tile_pool(name="candR", bufs=nbh) as crp, \
                 tc.tile_pool(name="tpf", bufs=8) as tpf, \
                 tc.tile_pool(name="enc_w", bufs=nd + 3) as wp, \
                 tc.tile_pool(name="enc_b", bufs=2) as bp, \
                 tc.tile_pool(name="enc_ast", bufs=3) as ap_, \
                 tc.tile_pool(name="psum_e", bufs=4, space="PSUM") as pse, \
                 tc.tile_pool(name="dec_g", bufs=4) as gp, \
                 tc.tile_pool(name="dec_sg", bufs=3) as sgp, \
                 tc.tile_pool(name="dec_acc", bufs=2) as accp:

                def encode_half(h, candV, candR, interleave=None):
                    """Emit encode+candidate ops for half h; pull from the
                    `interleave` generator (previous half's decode) after
                    each feature tile to keep the in-order engine queues
                    from head-blocking."""
                    bts = list(range(h * nbh, (h + 1) * nbh))
                    for ft in range(nft):
                        f0 = ft * FT
                        ws = []
                        for a in range(nd):
                            wst = wp.tile([PT, FT], F32R, tag="wst", name=f"wst{h}_{ft}_{a}")
                            nc.sync.dma_start(
                                out=wst[:, :], in_=wencT_r[:, a, f0 : f0 + FT]
                            )
                            ws.append(wst)
                        if with_bias:
                            bes = bp.tile([1, FT], F32R, tag="bes", name=f"bes{h}_{ft}")
                            nc.sync.dma_start(
                                out=bes[:, :],
                                in_=b_enc.rearrange("(o x) -> o x", o=1)[:, f0 : f0 + FT],
                            )
                        for bt in bts:
                            ps = pse.tile([PT, FT], F32, tag="pse", name=f"pse{h}_{ft}_{bt}")
                            for a in range(nd):
                                nc.tensor.matmul(
                                    ps[:, :],
                                    lhsT=xs[a][:, bt * PT : (bt + 1) * PT],
                                    rhs=ws[a][:, :],
                                    start=(a == 0),
                                    stop=(not with_bias) and (a == nd - 1),
                                )
                            if with_bias:
                                nc.tensor.matmul(
                                    ps[:, :], lhsT=ones[:, :], rhs=bes[:, :],
                                    start=False, stop=True,
                                )
                            ast = ap_.tile([PT, FT], F32, tag="ast", name=f"ast{h}_{ft}_{bt}")
                            nc.scalar.activation(
                                ast[:, :], ps[:, :], mybir.ActivationFunctionType.Relu
                            )
                            for g in range(FT // GRP):
                                c0 = (ft * (FT // GRP) + g) * 8
                                nc.vector.max(
                                    candV[bt][:, c0 : c0 + 8],
                                    ast[:, g * GRP : (g + 1) * GRP],
                                )
                                nc.vector.max_index(
                                    candR[bt][:, c0 : c0 + 8],
                                    candV[bt][:, c0 : c0 + 8],
                                    ast[:, g * GRP : (g + 1) * GRP],
                                )
                        if interleave is not None:
                            for _ in range(4):
                                next(interleave, None)

                def extract_half(h, candV, candR):
                    """Emit top-64 extraction for half h (all on DVE), eagerly
                    — before the next half's encode so the in-order DVE queue
                    never waits on ops behind it. Returns per-tile (t64, feat)."""
                    bts = list(range(h * nbh, (h + 1) * nbh))
                    res = {}
                    for bt in bts:
                        candPu = candV[bt][:, :].bitcast(U32)
                        nc.vector.tensor_scalar(
                            candPu, candPu, 0xFFFFFF00, None, mybir.AluOpType.bitwise_and
                        )
                        cr32 = tpf.tile([PT, ncand], U32, tag="cr32", name=f"cr32_{bt}", bufs=1)
                        nc.vector.tensor_copy(cr32[:, :], candR[bt][:, :])
                        nc.vector.tensor_tensor(
                            candPu, candPu, cr32[:, :], mybir.AluOpType.bitwise_or
                        )
                        t64 = tpf.tile([PT, K], F32, tag="t64", name=f"t64_{bt}", bufs=2 * nbh)
                        pos = tpf.tile([PT, K], U32, tag="pos", name=f"pos{bt}", bufs=2)
                        for r in range(nrounds):
                            t8 = t64[:, r * 8 : (r + 1) * 8]
                            nc.vector.max(t8, candV[bt][:, :])
                            nc.vector.max_index(
                                pos[:, r * 8 : (r + 1) * 8], t8, candV[bt][:, :]
                            )
                            if r < nrounds - 1:
                                nc.vector.match_replace(
                                    candV[bt][:, :], t8, candV[bt][:, :], -1e30
                                )
                        # feat = ((pos & ~7) << 5) | (bits(t64) & 0xFF)
                        feat = tpf.tile([PT, K], U32, tag="feat", name=f"feat{bt}", bufs=2 * nbh)
                        nc.vector.tensor_scalar(
                            feat[:, :], pos[:, :], 0xFFFFFFF8, 5,
                            mybir.AluOpType.bitwise_and,
                            mybir.AluOpType.logical_shift_left,
                        )
                        lowt = tpf.tile([PT, K], U32, tag="lowt", name=f"lowt{bt}", bufs=2)
                        nc.vector.tensor_scalar(
                            lowt[:, :], t64[:, :].bitcast(U32), 0xFF, None,
                            mybir.AluOpType.bitwise_and,
                        )
                        nc.vector.tensor_tensor(
                            feat[:, :], feat[:, :], lowt[:, :],
                            mybir.AluOpType.bitwise_or,
                        )
                        res[bt] = (t64, feat)
                    return res

                def decode_half(h, extracted, on_dve):
                    """Generator emitting the sparse gather decode for half h,
                    yielding after each gather/accumulate slot."""
                    bts = list(range(h * nbh, (h + 1) * nbh))
                    for bt in bts:
                        t64, feat = extracted[bt]
                        acc = accp.tile([PT, d], F32, tag="acc", name=f"acc{bt}")
                        for k in range(K):
                            G = gp.tile([PT, d], BF16, tag="g", name=f"g{bt}_{k}")
                            nc.gpsimd.indirect_dma_start(
                                out=G[:, :],
                                out_offset=None,
                                in_=wdec16[:, :],
                                in_offset=bass.IndirectOffsetOnAxis(
                                    ap=feat[:, k : k + 1], axis=0
                                ),
                            )
                            val = t64[:, k : k + 1]
                            if on_dve:
                                if k == 0:
                                    if with_bdec:
                                        nc.vector.scalar_tensor_tensor(
                                            acc[:, :], G[:, :], val, bdecb_sb[:, :],
                                            mybir.AluOpType.mult, mybir.AluOpType.add,
                                        )
                                    else:
                                        nc.vector.tensor_scalar(
                                            acc[:, :], G[:, :], val, None,
                                            mybir.AluOpType.mult,
                                        )
                                else:
                                    nc.vector.scalar_tensor_tensor(
                                        acc[:, :], G[:, :], val, acc[:, :],
                                        mybir.AluOpType.mult, mybir.AluOpType.add,
                                    )
                            else:
                                # Act scales into fp16, Pool accumulates
                                if k == 0:
                                    nc.scalar.activation(
                                        acc[:, :], G[:, :],
                                        mybir.ActivationFunctionType.Identity,
                                        scale=val,
                                    )
                                    if with_bdec:
                                        nc.gpsimd.tensor_tensor(
                                            acc[:, :], acc[:, :], bdecb_sb[:, :],
                                            mybir.AluOpType.add,
                                        )
                                else:
                                    sg = sgp.tile([PT, d], F16, tag="sg", name=f"sg{bt}_{k}")
                                    nc.scalar.activation(
                                        sg[:, :], G[:, :],
                                        mybir.ActivationFunctionType.Identity,
                                        scale=val,
                                    )
                                    nc.gpsimd.tensor_tensor(
                                        acc[:, :], acc[:, :], sg[:, :],
                                        mybir.AluOpType.add,
                                    )
                            yield
                        nc.sync.dma_start(out=out_r[bt, :, :], in_=acc[:, :])
                        yield

                prev_extracted = None
                for h in range(N_HALF):
                    bts = list(range(h * nbh, (h + 1) * nbh))
                    candV = {bt: cvp.tile([PT, ncand], F32, tag="cV", name=f"candV{bt}")
                             for bt in bts}
                    candR = {bt: crp.tile([PT, ncand], U16, tag="cR", name=f"candR{bt}")
                             for bt in bts}
                    prev = None
                    if prev_extracted is not None:
                        prev = decode_half(h - 1, prev_extracted, on_dve=False)
                    encode_half(h, candV, candR, interleave=prev)
                    if prev is not None:
                        for _ in prev:
                            pass
                    prev_extracted = extract_half(h, candV, candR)
                # final half's decode on DVE
                for _ in decode_half(N_HALF - 1, prev_extracted, on_dve=True):
                    pass

    split_waits(nc)
    return nc


def kernel(x, W_enc, b_enc, W_dec, b_dec):
    b, d = x.shape
    f = W_enc.shape[0]
    b_loc = b // N_CORES

    xT = np.ascontiguousarray(np.asarray(x, dtype=np.float32).T)       # [d, b]
    wenc = np.asarray(W_enc, dtype=np.float32)
    wencT = np.ascontiguousarray(wenc.T)                               # [d, f]
    wdec16 = np.ascontiguousarray(
        np.asarray(W_dec, dtype=np.float32).T.astype(ml_dtypes.bfloat16)
    )  # [f, d] bf16
    bdec = np.asarray(b_dec, dtype=np.float32)
    # fold the x - b_dec subtraction into the encoder bias
    benc_eff = np.asarray(b_enc, dtype=np.float32) - wenc @ bdec
    with_bias = bool(np.any(benc_eff))
    with_bdec = bool(np.any(bdec))

    nc = build_nc(b_loc, d, f, with_bias, with_bdec)

    in_maps = []
    for i in range(N_CORES):
        m = {
            "xT": np.ascontiguousarray(xT[:, i * b_loc : (i + 1) * b_loc]),
            "W_encT": wencT,
            "Wdec16": wdec16,
        }
        if with_bias:
            m["b_enc"] = benc_eff
        if with_bdec:
            m["bdecb"] = np.ascontiguousarray(np.broadcast_to(bdec, (PT, d)))
        in_maps.append(m)

    trace = bool(os.environ.get("BASS_TOPK_TRACE"))
    res = run_bass_kernel_spmd(nc, in_maps, list(range(N_CORES)), trace=trace)
    if trace and res.exec_time_ns is not None:
        print(f"HW exec time: {res.exec_time_ns} ns")
        if res.instructions_and_trace is not None:
            print(f"trace path: {res.instructions_and_trace[1]}")
        if res.profile_json is not None:
            print(f"profile json: {res.profile_json}")
    shards = [res.results[i]["out"] for i in range(N_CORES)]     # [b_loc, d] each
    return np.ascontiguousarray(np.concatenate(shards, axis=0))


if __name__ == "__main__":
    # small smoke config vs numpy simulation of the same math
    b_loc, d, f = 256, 256, 2048
    rng = np.random.default_rng(0)
    x = rng.standard_normal((N_CORES * b_loc, d), dtype=np.float32)
    W_enc = (rng.standard_normal((f, d), dtype=np.float32) / np.sqrt(d)).astype(np.float32)
    W_dec = rng.standard_normal((d, f), dtype=np.float32).astype(np.float32)

    import sys
    if "zeros" in sys.argv[1:]:
        b_enc_ = np.zeros(f, dtype=np.float32)
        b_dec_ = np.zeros(d, dtype=np.float32)
    else:
        b_enc_ = rng.standard_normal(f, dtype=np.float32) * 0.01
        b_dec_ = rng.standard_normal(d, dtype=np.float32) * 0.01

    got = kernel(x, W_enc, b_enc_, W_dec, b_dec_)

    pre = (x - b_dec_) @ W_enc.T + b_enc_
    acts = np.maximum(pre, 0)
    # simulate the kernel's group-candidate top-64 (with packed low bits)
    g = acts.reshape(acts.shape[0], -1, GRP)
    order = np.argsort(-g, axis=2, kind="stable")[:, :, :8]
    top8v = np.take_along_axis(g, order, axis=2)
    packed = ((top8v.view(np.uint32) & 0xFFFFFF00) | order.astype(np.uint32)).view(np.float32)
    flat = packed.reshape(acts.shape[0], -1)
    srt = np.argsort(-flat, axis=1, kind="stable")[:, :K]
    vals = np.take_along_axis(flat, srt, axis=1)
    feats = ((srt & ~7) << 5) | (vals.view(np.uint32) & 0xFF)
    wd16 = W_dec.T.astype(ml_dtypes.bfloat16).astype(np.float32)  # [f, d]
    want = np.einsum("bk,bkd->bd", vals, wd16[feats]) + b_dec_
    err = np.linalg.norm(got - want) / np.linalg.norm(want)
    print("smoke rel err:", err)


# revision 22
# speedup vs baseline: 1.5554x; 1.0167x over previous
"""TopK sparse autoencoder forward pass on 8 TRN2 NeuronCores.

Strategy: data-parallel over the token batch (8192 rows -> 1024 rows/core,
zero collectives). Per core, the batch is processed in two halves so the
second half's encode overlaps the first half's sparse decode:
  1. encode: pre = x @ W_enc.T + b_eff as fp32r matmuls at full PE rate
     (batch rows on partitions, features on the free dim); the x - b_dec
     subtraction is folded into b_eff host-side. ReLU fused into the
     PSUM->SBUF copy on the Activation engine.
  2. top-64 with indices: per 256-wide feature group, DVE Max8 + MaxIndex
     give the top-8 values and their within-group positions; the position is
     packed into the low 8 mantissa bits of the value (2^-15 relative
     perturbation, far below fp32r noise). 8 rounds of Max8 + MaxIndex +
     MatchReplace over the 1152 packed candidates yield the top-64 packed
     values and their candidate positions; the feature index is recovered
     arithmetically: feat = 256*(pos>>3) | (bits(val) & 0xFF).
  3. sparse decode: for each of the 64 slots, an indirect DMA gathers
     W_dec^T rows (bf16) by feature index, one row per token partition.
     Half A (overlapped with half B's encode): Act scales G*val -> fp16,
     Pool accumulates — DVE stays free for half B's candidate scan, and
     the ops are emission-interleaved into the encode loop to avoid
     head-of-line blocking on the in-order engine queues. The final half
     accumulates with DVE scalar_tensor_tensor (Pool is busy generating
     gather descriptors).
"""

import os
import numpy as np
import ml_dtypes

from concourse import bass, mybir
from concourse import tile
from concourse.bass_utils import run_bass_kernel_spmd

F32 = mybir.dt.float32
F32R = mybir.dt.float32r
F16 = mybir.dt.float16
U32 = mybir.dt.uint32
U16 = mybir.dt.uint16
BF16 = mybir.dt.bfloat16

N_CORES = 8
B, D, F, K = 8192, 2304, 36864, 64

PT = 128           # partition tile
FT = 512           # encode feature tile (matmul moving dim)
GRP = 256          # max8 candidate group width
N_HALF = 2         # batch halves for encode/decode overlap


def split_waits(nc, maxw=1):
    """Walrus in this container accepts few sync-waits per instruction; Tile
    emits many. Move excess waits onto standalone same-engine no-ops."""
    for fn in nc.m.functions:
        for blk in fn.blocks:
            newinsts = []
            for inst in blk.instructions:
                si = inst.sync_info
                if si is not None and len(si.on_wait) > maxw:
                    extra = si.on_wait[:-maxw]
                    keep = si.on_wait[-maxw:]
                    for j, w in enumerate(extra):
                        nop = mybir.InstNoOp(name=f"{inst.name}-wsplit{j}", ins=[], outs=[])
                        nop.engine = inst.engine
                        nop.sync_info = mybir.SyncInfo(on_wait=[w], on_update=[])
                        newinsts.append(nop)
                    si.on_wait = keep
                newinsts.append(inst)
            blk.instructions = newinsts


def build_nc(b_loc, d, f, with_bias, with_bdec):
    nbt = b_loc // PT          # batch tiles
    nd = d // PT               # contraction chunks
    nft = f // FT              # encode feature tiles
    ngrp = f // GRP            # candidate groups per row
    ncand = ngrp * 8           # candidates per row
    assert ncand >= K and K % 8 == 0
    nrounds = K // 8
    assert nbt % N_HALF == 0
    nbh = nbt // N_HALF        # batch tiles per half

    nc = bass.Bass()
    # x and weights declared float32r: same 4-byte container as f32, PE
    # rounds internally — lets plain DMAs satisfy the BIR fp32r-producer
    # rule with zero cast instructions.
    xT = nc.declare_dram_parameter("xT", [d, b_loc], F32R, isOutput=False)
    wencT = nc.declare_dram_parameter("W_encT", [d, f], F32R, isOutput=False)
    wdec16 = nc.declare_dram_parameter("Wdec16", [f, d], BF16, isOutput=False)
    if with_bias:
        b_enc = nc.declare_dram_parameter("b_enc", [f], F32R, isOutput=False)
    if with_bdec:
        bdecb = nc.declare_dram_parameter("bdecb", [PT, d], F32, isOutput=False)
    out = nc.declare_dram_parameter("out", [b_loc, d], F32, isOutput=True)

    wencT_r = wencT.rearrange("(a p) f -> p a f", p=PT)   # [128, nd, f]
    xT_r = xT.rearrange("(a p) b -> p a b", p=PT)         # [128, nd, b_loc]
    out_r = out.rearrange("(t p) e -> t p e", p=PT)       # [nbt, 128, d]

    with tile.TileContext(nc) as tc:
        with tc.tile_pool(name="persist", bufs=1) as pp:
            if with_bias:
                ones_st = pp.tile([1, PT], F32)
                nc.vector.memset(ones_st[:, :], 1.0)
                ones = pp.tile([1, PT], F32R)
                nc.vector.tensor_copy(ones[:, :], ones_st[:, :])
            if with_bdec:
                bdecb_sb = pp.tile([PT, d], F32)
                nc.sync.dma_start(out=bdecb_sb[:, :], in_=bdecb[:, :])

            # x tiles, resident for the whole encode
            xs = []
            for a in range(nd):
                xt = pp.tile([PT, b_loc], F32R, name=f"xs{a}")
                nc.sync.dma_start(out=xt[:, :], in_=xT_r[:, a, :])
                xs.append(xt)

            with tc.tile_pool(name="candV", bufs=nbh) as cvp, \
                 tc.tile_pool(name="candR", bufs=nbh) as crp, \
                 tc.tile_pool(name="tpf", bufs=8) as tpf, \
                 tc.tile_pool(name="enc_w", bufs=nd + 3) as wp, \
                 tc.tile_pool(name="enc_b", bufs=2) as bp, \
                 tc.tile_pool(name="enc_ast", bufs=3) as ap_, \
                 tc.tile_pool(name="psum_e", bufs=4, space="PSUM") as pse, \
                 tc.tile_pool(name="dec_g", bufs=4) as gp, \
                 tc.tile_pool(name="dec_sg", bufs=3) as sgp, \
                 tc.tile_pool(name="dec_acc", bufs=2) as accp:

                def encode_half(h, candV, candR, interleave=None):
                    """Emit encode+candidate ops for half h; pull from the
                    `interleave` generator (previous half's decode) after
                    each feature tile to keep the in-order engine queues
                    from head-blocking."""
                    bts = list(range(h * nbh, (h + 1) * nbh))
                    for ft in range(nft):
                        f0 = ft * FT
                        ws = []
                        for a in range(nd):
                            wst = wp.tile([PT, FT], F32R, tag="wst", name=f"wst{h}_{ft}_{a}")
                            nc.sync.dma_start(
                                out=wst[:, :], in_=wencT_r[:, a, f0 : f0 + FT]
                            )
                            ws.append(wst)
                        if with_bias:
                            bes = bp.tile([1, FT], F32R, tag="bes", name=f"bes{h}_{ft}")
                            nc.sync.dma_start(
                                out=bes[:, :],
                                in_=b_enc.rearrange("(o x) -> o x", o=1)[:, f0 : f0 + FT],
                            )
                        for bt in bts:
                            ps = pse.tile([PT, FT], F32, tag="pse", name=f"pse{h}_{ft}_{bt}")
                            for a in range(nd):
                                nc.tensor.matmul(
                                    ps[:, :],
                                    lhsT=xs[a][:, bt * PT : (bt + 1) * PT],
                                    rhs=ws[a][:, :],
                                    start=(a == 0),
                                    stop=(not with_bias) and (a == nd - 1),
                                )
                            if with_bias:
                                nc.tensor.matmul(
                                    ps[:, :], lhsT=ones[:, :], rhs=bes[:, :],
                                    start=False, stop=True,
                                )
                            ast = ap_.tile([PT, FT], F32, tag="ast", name=f"ast{h}_{ft}_{bt}")
                            nc.scalar.activation(
                                ast[:, :], ps[:, :], mybir.ActivationFunctionType.Relu
                            )
                            for g in range(FT // GRP):
                                c0 = (ft * (FT // GRP) + g) * 8
                                nc.vector.max(
                                    candV[bt][:, c0 : c0 + 8],
                                    ast[:, g * GRP : (g + 1) * GRP],
                                )
                                nc.vector.max_index(
                                    candR[bt][:, c0 : c0 + 8],
                                    candV[bt][:, c0 : c0 + 8],
                                    ast[:, g * GRP : (g + 1) * GRP],
                                )
                        if interleave is not None:
                            for _ in range(4):
                                next(interleave, None)

                def extract_half(h, candV, candR):
                    """Emit top-64 extraction for half h (all on DVE), eagerly
                    — before the next half's encode so the in-order DVE queue
                    never waits on ops behind it. Returns per-tile (t64, feat)."""
                    bts = list(range(h * nbh, (h + 1) * nbh))
                    res = {}
                    for bt in bts:
                        candPu = candV[bt][:, :].bitcast(U32)
                        nc.vector.tensor_scalar(
                            candPu, candPu, 0xFFFFFF00, None, mybir.AluOpType.bitwise_and
                        )
                        cr32 = tpf.tile([PT, ncand], U32, tag="cr32", name=f"cr32_{bt}", bufs=1)
                        nc.vector.tensor_copy(cr32[:, :], candR[bt][:, :])
                        nc.vector.tensor_tensor(
                            candPu, candPu, cr32[:, :], mybir.AluOpType.bitwise_or
                        )
                        t64 = tpf.tile([PT, K], F32, tag="t64", name=f"t64_{bt}", bufs=2 * nbh)
                        pos = tpf.tile([PT, K], U32, tag="pos", name=f"pos{bt}", bufs=2)
                        for r in range(nrounds):
                            t8 = t64[:, r * 8 : (r + 1) * 8]
                            nc.vector.max(t8, candV[bt][:, :])
                            nc.vector.max_index(
                                pos[:, r * 8 : (r + 1) * 8], t8, candV[bt][:, :]
                            )
                            if r < nrounds - 1:
                                nc.vector.match_replace(
                                    candV[bt][:, :], t8, candV[bt][:, :], -1e30
                                )
                        # feat = ((pos & ~7) << 5) | (bits(t64) & 0xFF)
                        feat = tpf.tile([PT, K], U32, tag="feat", name=f"feat{bt}", bufs=2 * nbh)
                        nc.vector.tensor_scalar(
                            feat[:, :], pos[:, :], 0xFFFFFFF8, 5,
                            mybir.AluOpType.bitwise_and,
                            mybir.AluOpType.logical_shift_left,
                        )
                        lowt = tpf.tile([PT, K], U32, tag="lowt", name=f"lowt{bt}", bufs=2)
                        nc.vector.tensor_scalar(
                            lowt[:, :], t64[:, :].bitcast(U32), 0xFF, None,
                            mybir.AluOpType.bitwise_and,
                        )
                        nc.vector.tensor_tensor(
                            feat[:, :], feat[:, :], lowt[:, :],
                            mybir.AluOpType.bitwise_or,
                        )
                        res[bt] = (t64, feat)
                    return res

                def decode_half(h, extracted):
                    """Generator emitting the sparse gather decode for half h,
                    yielding after each gather/accumulate slot. Pool generates
                    gather descriptors, Act scales G*val into fp16, DVE
                    accumulates fp16+fp16 in 2X mode."""
                    bts = list(range(h * nbh, (h + 1) * nbh))
                    for bt in bts:
                        t64, feat = extracted[bt]
                        acc16 = accp.tile([PT, d], F16, tag="acc16", name=f"acc16_{bt}")
                        for k in range(K):
                            G = gp.tile([PT, d], BF16, tag="g", name=f"g{bt}_{k}")
                            nc.gpsimd.indirect_dma_start(
                                out=G[:, :],
                                out_offset=None,
                                in_=wdec16[:, :],
                                in_offset=bass.IndirectOffsetOnAxis(
                                    ap=feat[:, k : k + 1], axis=0
                                ),
                            )
                            val = t64[:, k : k + 1]
                            if k == 0:
                                nc.scalar.activation(
                                    acc16[:, :], G[:, :],
                                    mybir.ActivationFunctionType.Identity,
                                    scale=val,
                                )
                            else:
                                sg = sgp.tile([PT, d], F16, tag="sg", name=f"sg{bt}_{k}")
                                nc.scalar.activation(
                                    sg[:, :], G[:, :],
                                    mybir.ActivationFunctionType.Identity,
                                    scale=val,
                                )
                                nc.vector.tensor_tensor(
                                    acc16[:, :], acc16[:, :], sg[:, :],
                                    mybir.AluOpType.add,
                                )
                            yield
                        acc32 = accp.tile([PT, d], F32, tag="acc32", name=f"acc32_{bt}", bufs=1)
                        if with_bdec:
                            nc.vector.tensor_tensor(
                                acc32[:, :], acc16[:, :], bdecb_sb[:, :],
                                mybir.AluOpType.add,
                            )
                        else:
                            nc.scalar.activation(
                                acc32[:, :], acc16[:, :],
                                mybir.ActivationFunctionType.Identity,
                            )
                        nc.sync.dma_start(out=out_r[bt, :, :], in_=acc32[:, :])
                        yield

                prev_extracted = None
                for h in range(N_HALF):
                    bts = list(range(h * nbh, (h + 1) * nbh))
                    candV = {bt: cvp.tile([PT, ncand], F32, tag="cV", name=f"candV{bt}")
                             for bt in bts}
                    candR = {bt: crp.tile([PT, ncand], U16, tag="cR", name=f"candR{bt}")
                             for bt in bts}
                    prev = None
                    if prev_extracted is not None:
                        prev = decode_half(h - 1, prev_extracted)
                    encode_half(h, candV, candR, interleave=prev)
                    if prev is not None:
                        for _ in prev:
                            pass
                    prev_extracted = extract_half(h, candV, candR)
                # final half's decode
                for _ in decode_half(N_HALF - 1, prev_extracted):
                    pass

    split_waits(nc)
    return nc


def kernel(x, W_enc, b_enc, W_dec, b_dec):
    b, d = x.shape
    f = W_enc.shape[0]
    b_loc = b // N_CORES

    xT = np.ascontiguousarray(np.asarray(x, dtype=np.float32).T)       # [d, b]
    wenc = np.asarray(W_enc, dtype=np.float32)
    wencT = np.ascontiguousarray(wenc.T)                               # [d, f]
    wdec16 = np.ascontiguousarray(
        np.asarray(W_dec, dtype=np.float32).T.astype(ml_dtypes.bfloat16)
    )  # [f, d] bf16
    bdec = np.asarray(b_dec, dtype=np.float32)
    # fold the x - b_dec subtraction into the encoder bias
    benc_eff = np.asarray(b_enc, dtype=np.float32) - wenc @ bdec
    with_bias = bool(np.any(benc_eff))
    with_bdec = bool(np.any(bdec))

    nc = build_nc(b_loc, d, f, with_bias, with_bdec)

    in_maps = []
    for i in range(N_CORES):
        m = {
            "xT": np.ascontiguousarray(xT[:, i * b_loc : (i + 1) * b_loc]),
            "W_encT": wencT,
            "Wdec16": wdec16,
        }
        if with_bias:
            m["b_enc"] = benc_eff
        if with_bdec:
            m["bdecb"] = np.ascontiguousarray(np.broadcast_to(bdec, (PT, d)))
        in_maps.append(m)

    trace = bool(os.environ.get("BASS_TOPK_TRACE"))
    res = run_bass_kernel_spmd(nc, in_maps, list(range(N_CORES)), trace=trace)
    if trace and res.exec_time_ns is not None:
        print(f"HW exec time: {res.exec_time_ns} ns")
        if res.instructions_and_trace is not None:
            print(f"trace path: {res.instructions_and_trace[1]}")
        if res.profile_json is not None:
            print(f"profile json: {res.profile_json}")
    shards = [res.results[i]["out"] for i in range(N_CORES)]     # [b_loc, d] each
    return np.ascontiguousarray(np.concatenate(shards, axis=0))


if __name__ == "__main__":
    # small smoke config vs numpy simulation of the same math
    b_loc, d, f = 256, 256, 2048
    rng = np.random.default_rng(0)
    x = rng.standard_normal((N_CORES * b_loc, d), dtype=np.float32)
    W_enc = (rng.standard_normal((f, d), dtype=np.float32) / np.sqrt(d)).astype(np.float32)
    W_dec = rng.standard_normal((d, f), dtype=np.float32).astype(np.float32)

    import sys
    if "zeros" in sys.argv[1:]:
        b_enc_ = np.zeros(f, dtype=np.float32)
        b_dec_ = np.zeros(d, dtype=np.float32)
    else:
        b_enc_ = rng.standard_normal(f, dtype=np.float32) * 0.01
        b_dec_ = rng.standard_normal(d, dtype=np.float32) * 0.01

    got = kernel(x, W_enc, b_enc_, W_dec, b_dec_)

    pre = (x - b_dec_) @ W_enc.T + b_enc_
    acts = np.maximum(pre, 0)
    # simulate the kernel's group-candidate top-64 (with packed low bits)
    g = acts.reshape(acts.shape[0], -1, GRP)
    order = np.argsort(-g, axis=2, kind="stable")[:, :, :8]
    top8v = np.take_along_axis(g, order, axis=2)
    packed = ((top8v.view(np.uint32) & 0xFFFFFF00) | order.astype(np.uint32)).view(np.float32)
    flat = packed.reshape(acts.shape[0], -1)
    srt = np.argsort(-flat, axis=1, kind="stable")[:, :K]
    vals = np.take_along_axis(flat, srt, axis=1)
    feats = ((srt & ~7) << 5) | (vals.view(np.uint32) & 0xFF)
    wd16 = W_dec.T.astype(ml_dtypes.bfloat16).astype(np.float32)  # [f, d]
    want = np.einsum("bk,bkd->bd", vals, wd16[feats]) + b_dec_
    err = np.linalg.norm(got - want) / np.linalg.norm(want)
    print("smoke rel err:", err)
